# revision 5
# baseline (speedup 1.0000x reference)
"""Single-head causal attention (B=16, T=2048, E=384, H=64) on 8 NeuronCores.

Data-parallel over batch B across the 8 cores (2 batches per core); the tiny
W_qkv is replicated. Hand-written Bass/Tile kernel (bacc.Bacc + TileContext;
nc.compile() runs the wait-splitting legalization that walrus needs).

Per core, per batch:
  1. x tiles [128,384] f32 DMA'd in, cast to bf16 (DVE), PE-transposed to
     x^T bf16 [3x[128,2048]] (PSUM roundtrip).
  2. qkv^T = W^T @ x^T on PE (lhsT = W bf16), PSUM->SBUF bf16.
     Rows 0:64 = q^T, 64:128 = k^T, v^T separate [64,2048].
  3. v^T PE-transposed back to v natural [128, 16*65] with a ones column per
     128-key tile (row-sum trick).
  4. Per 512-query chunk c, per 128-key tile m <= 4c+3:
       S^T = k_m @ q^T  (PSUM [128,512], keys on partitions)
       P^T = exp(S^T/8) (ScalarE, bf16, causal mask = precomputed mask mul on
       diagonal-band tiles), PV: out^T += v_m^T . P^T accumulated in PSUM
       ([65,512]; row 64 = softmax denominator).
  5. out^T -> bf16, PE-transpose [65,128] blocks -> [128,65], reciprocal of
     col 64, tensor_scalar mul, DMA [128,64] f32 to HBM.
"""

import sys

import numpy as np

B, T, E, H = 16, 2048, 384, 64
N_CORES = 8
BPC = B // N_CORES  # batches per core
NT = T // 128       # 16 row tiles per batch
NC = T // 512       # 4 query chunks per batch

_cache = {}


def _ensure_path():
    if "/opt/trn_rl_repo" not in sys.path:
        sys.path.insert(0, "/opt/trn_rl_repo")


def _build():
    """Build + compile the per-core Bass program. Returns nc."""
    _ensure_path()
    from contextlib import ExitStack

    import concourse.bass as bass
    import concourse.tile as tile
    from concourse import bacc, mybir

    f32 = mybir.dt.float32
    bf16 = mybir.dt.bfloat16

    nc = bacc.Bacc("TRN2", target_bir_lowering=False, debug=False,
                   num_devices=N_CORES)

    x_in = nc.declare_dram_parameter("x", [BPC, T, E], f32, isOutput=False)
    w_in = nc.declare_dram_parameter("w", [E, 3 * H], bf16, isOutput=False)
    id_in = nc.declare_dram_parameter("ident", [128, 128], bf16, isOutput=False)
    mk_in = nc.declare_dram_parameter("mask", [128, 896], bf16, isOutput=False)
    out_t = nc.declare_dram_parameter("out", [BPC, T, H], f32, isOutput=True)

    with tile.TileContext(nc) as tc, ExitStack() as ctx:
        const_p = ctx.enter_context(tc.tile_pool(name="const", bufs=1))
        xn_p = ctx.enter_context(tc.tile_pool(name="xn", bufs=3))
        xb_p = ctx.enter_context(tc.tile_pool(name="xb", bufs=3))
        xt_p = ctx.enter_context(tc.tile_pool(name="xt", bufs=6))
        qk_p = ctx.enter_context(tc.tile_pool(name="qk", bufs=2))
        vt_p = ctx.enter_context(tc.tile_pool(name="vt", bufs=2))
        vn_p = ctx.enter_context(tc.tile_pool(name="vn", bufs=2))
        pt_p = ctx.enter_context(tc.tile_pool(name="pt", bufs=4))
        ot_p = ctx.enter_context(tc.tile_pool(name="ot", bufs=2))
        rc_p = ctx.enter_context(tc.tile_pool(name="rc", bufs=3))
        ob_p = ctx.enter_context(tc.tile_pool(name="ob", bufs=3))
        ps_tr = ctx.enter_context(tc.tile_pool(name="ps_tr", bufs=2, space="PSUM"))
        ps_mm = ctx.enter_context(tc.tile_pool(name="ps_mm", bufs=2, space="PSUM"))
        ps_out = ctx.enter_context(tc.tile_pool(name="ps_out", bufs=2, space="PSUM"))
        ps_otr = ctx.enter_context(tc.tile_pool(name="ps_otr", bufs=2, space="PSUM"))

        # constants (once per core)
        ident = const_p.tile([128, 128], bf16, tag="ident")
        nc.sync.dma_start(ident[:], id_in[:])
        mask = const_p.tile([128, 896], bf16, tag="mask")
        nc.sync.dma_start(mask[:], mk_in[:])
        w_sb = []
        for k in range(3):
            wt = const_p.tile([128, 3 * H], bf16, tag=f"w{k}")
            nc.sync.dma_start(wt[:], w_in[128 * k:128 * (k + 1), :])
            w_sb.append(wt)

        for b in range(BPC):
            # ---- Phase A: load x, cast, transpose to x^T ----
            xt = [xt_p.tile([128, T], bf16, tag="xt", name=f"xt{b}_{k}")
                  for k in range(3)]
            for i in range(NT):
                xn = xn_p.tile([128, E], f32, tag="xn")
                nc.sync.dma_start(xn[:], x_in[b, 128 * i:128 * (i + 1), :])
                xb = xb_p.tile([128, E], bf16, tag="xb")
                nc.vector.tensor_copy(xb[:], xn[:])
                for k in range(3):
                    pst = ps_tr.tile([128, 128], bf16, tag="tr")
                    nc.tensor.transpose(pst[:], xb[:, 128 * k:128 * (k + 1)],
                                        ident[:])
                    nc.vector.tensor_copy(xt[k][:, 128 * i:128 * (i + 1)], pst[:])

            # ---- qkv^T = W^T @ x^T (three M=64 groups, all base partition 0) ----
            qT = qk_p.tile([64, T], bf16, tag="qT")
            kT = qk_p.tile([64, T], bf16, tag="kT")
            vT = vt_p.tile([64, T], bf16, tag="vt")
            dsts = (qT, kT, vT)
            for mt in range(3):
                msl = slice(64 * mt, 64 * (mt + 1))
                for n in range(4):
                    nsl = slice(512 * n, 512 * (n + 1))
                    ps = ps_mm.tile([128, 512], f32, tag="mm")
                    for k in range(3):
                        nc.tensor.matmul(ps[0:64, :], w_sb[k][:, msl],
                                         xt[k][:, nsl],
                                         start=(k == 0), stop=(k == 2))
                    nc.scalar.copy(dsts[mt][:, nsl], ps[0:64, :])

            # ---- v natural [128, 16*65] with ones column per tile ----
            v_nat = vn_p.tile([128, 65 * NT], bf16, tag="vn")
            nc.gpsimd.memset(v_nat[:], 1.0)
            for m in range(NT):
                pst = ps_tr.tile([128, 64], bf16, tag="tr")
                nc.tensor.transpose(pst[:], vT[:, 128 * m:128 * (m + 1)],
                                    ident[0:64, 0:64])
                nc.vector.tensor_copy(v_nat[:, 65 * m:65 * m + 64], pst[:])

            # ---- attention per 512-query chunk ----
            for c in range(NC):
                nm = 4 * c + 4  # key tiles 0..nm-1
                cs = slice(512 * c, 512 * (c + 1))
                ps_o = ps_out.tile([65, 512], f32, tag="out")
                prev_pt = None
                for m in range(nm):
                    ps_s = ps_mm.tile([128, 512], f32, tag="mm")
                    nc.tensor.matmul(ps_s[:, :],
                                     kT[:, 128 * m:128 * (m + 1)],
                                     qT[:, cs], start=True, stop=True)
                    pt = pt_p.tile([128, 512], bf16, tag="pt")
                    nc.scalar.activation(pt[:], ps_s[:, :],
                                         mybir.ActivationFunctionType.Exp,
                                         scale=0.125)
                    d = m - 4 * c
                    if d >= 0:
                        off = 384 - 128 * d
                        nc.vector.tensor_mul(pt[:], pt[:],
                                             mask[:, off:off + 512])
                    # software-pipelined PV (one behind)
                    if prev_pt is not None:
                        pm = m - 1
                        nc.tensor.matmul(ps_o[:, :],
                                         v_nat[:, 65 * pm:65 * pm + 65],
                                         prev_pt[:],
                                         start=(pm == 0), stop=False)
                    prev_pt = pt
                pm = nm - 1
                nc.tensor.matmul(ps_o[:, :], v_nat[:, 65 * pm:65 * pm + 65],
                                 prev_pt[:], start=(pm == 0), stop=True)

                # ---- epilogue: normalize + transpose back + store ----
                otb = ot_p.tile([65, 512], bf16, tag="ot")
                nc.scalar.copy(otb[:], ps_o[:, :])
                for j in range(4):
                    pso = ps_otr.tile([128, 65], bf16, tag="otr")
                    nc.tensor.transpose(pso[:], otb[:, 128 * j:128 * (j + 1)],
                                        ident[0:65, 0:65])
                    rc = rc_p.tile([128, 1], f32, tag="rc")
                    nc.vector.reciprocal(rc[:], pso[:, 64:65])
                    ob = ob_p.tile([128, H], f32, tag="ob")
                    nc.vector.tensor_scalar_mul(ob[:], pso[:, 0:64], rc[:])
                    t0 = 512 * c + 128 * j
                    nc.sync.dma_start(out_t[b, t0:t0 + 128, :], ob[:])

    nc.compile()
    return nc


def _get_nc():
    if "nc" not in _cache:
        _cache["nc"] = _build()
    return _cache["nc"]


def _host_inputs(W_qkv):
    _ensure_path()
    import ml_dtypes

    w_bf = np.ascontiguousarray(W_qkv.astype(ml_dtypes.bfloat16))
    ident = np.eye(128, dtype=ml_dtypes.bfloat16)
    # mask[j, u] = 1 iff u >= j + 384  (sliced per diagonal offset d)
    j = np.arange(128)[:, None]
    u = np.arange(896)[None, :]
    mask = (u >= j + 384).astype(ml_dtypes.bfloat16)
    return w_bf, ident, mask


def kernel(x: np.ndarray, W_qkv: np.ndarray) -> np.ndarray:
    _ensure_path()
    from concourse.bass_utils import run_bass_kernel_spmd

    nc = _get_nc()
    x = np.ascontiguousarray(x, dtype=np.float32)
    w_bf, ident, mask = _host_inputs(np.asarray(W_qkv, dtype=np.float32))
    xs = x.reshape(N_CORES, BPC, T, E)
    in_maps = [
        {"x": xs[c], "w": w_bf, "ident": ident, "mask": mask}
        for c in range(N_CORES)
    ]
    res = run_bass_kernel_spmd(nc, in_maps, list(range(N_CORES)))
    out = np.stack([res.results[c]["out"] for c in range(N_CORES)], axis=0)
    return out.reshape(B, T, H).astype(np.float32)


if __name__ == "__main__":
    rng = np.random.default_rng(0)
    x = rng.standard_normal((B, T, E), dtype=np.float32)
    W = rng.standard_normal((E, 3 * H), dtype=np.float32) * (E ** -0.5)
    out = kernel(x=x, W_qkv=W)
    print("out", out.shape, out.dtype, float(np.abs(out).max()))


# revision 8
# speedup vs baseline: 455.7611x; 455.7611x over previous
"""Single-head causal attention (B=16, T=2048, E=384, H=64) on 8 NeuronCores.

Data-parallel over batch B across the 8 cores (2 batches per core); the tiny
W_qkv is replicated. Hand-written Bass/Tile kernel (bacc.Bacc + TileContext;
nc.compile() runs the wait-splitting legalization that walrus needs).

Per core, per batch:
  1. x tiles [128,384] f32 DMA'd in, cast to bf16 (DVE), PE-transposed to
     x^T bf16 [3x[128,2048]] (PSUM roundtrip).
  2. qkv^T = W^T @ x^T on PE (lhsT = W bf16), PSUM->SBUF bf16.
     Rows 0:64 = q^T, 64:128 = k^T, v^T separate [64,2048].
  3. v^T PE-transposed back to v natural [128, 16*65] with a ones column per
     128-key tile (row-sum trick).
  4. Per 512-query chunk c, per 128-key tile m <= 4c+3:
       S^T = k_m @ q^T  (PSUM [128,512], keys on partitions)
       P^T = exp(S^T/8) (ScalarE, bf16, causal mask = precomputed mask mul on
       diagonal-band tiles), PV: out^T += v_m^T . P^T accumulated in PSUM
       ([65,512]; row 64 = softmax denominator).
  5. out^T -> bf16, PE-transpose [65,128] blocks -> [128,65], reciprocal of
     col 64, tensor_scalar mul, DMA [128,64] f32 to HBM.
"""

import sys

import numpy as np

B, T, E, H = 16, 2048, 384, 64
N_CORES = 8
BPC = B // N_CORES  # batches per core
NT = T // 128       # 16 row tiles per batch
NC = T // 512       # 4 query chunks per batch

_cache = {}


def _ensure_path():
    if "/opt/trn_rl_repo" not in sys.path:
        sys.path.insert(0, "/opt/trn_rl_repo")


def _build(loop_n=None):
    """Build + compile the per-core Bass program. Returns nc.

    loop_n: if set, wrap the whole body in an on-device For_i loop that
    repeats the identical computation loop_n times — used by test.py to
    measure marginal per-iteration HW time past the fixed axon dispatch
    floor (~68ms/call)."""
    _ensure_path()
    from contextlib import ExitStack

    import concourse.bass as bass
    import concourse.tile as tile
    from concourse import bacc, mybir

    f32 = mybir.dt.float32
    bf16 = mybir.dt.bfloat16

    nc = bacc.Bacc("TRN2", target_bir_lowering=False, debug=False,
                   num_devices=N_CORES)

    x_in = nc.declare_dram_parameter("x", [BPC, T, E], f32, isOutput=False)
    w_in = nc.declare_dram_parameter("w", [E, 3 * H], bf16, isOutput=False)
    id_in = nc.declare_dram_parameter("ident", [128, 128], bf16, isOutput=False)
    mk_in = nc.declare_dram_parameter("mask", [128, 896], bf16, isOutput=False)
    out_t = nc.declare_dram_parameter("out", [BPC, T, H], f32, isOutput=True)

    with tile.TileContext(nc) as tc, ExitStack() as ctx:
        const_p = ctx.enter_context(tc.tile_pool(name="const", bufs=1))
        xn_p = ctx.enter_context(tc.tile_pool(name="xn", bufs=3))
        xb_p = ctx.enter_context(tc.tile_pool(name="xb", bufs=3))
        xt_p = ctx.enter_context(tc.tile_pool(name="xt", bufs=6))
        qk_p = ctx.enter_context(tc.tile_pool(name="qk", bufs=2))
        vt_p = ctx.enter_context(tc.tile_pool(name="vt", bufs=2))
        vn_p = ctx.enter_context(tc.tile_pool(name="vn", bufs=2))
        pt_p = ctx.enter_context(tc.tile_pool(name="pt", bufs=4))
        ot_p = ctx.enter_context(tc.tile_pool(name="ot", bufs=2))
        rc_p = ctx.enter_context(tc.tile_pool(name="rc", bufs=3))
        ob_p = ctx.enter_context(tc.tile_pool(name="ob", bufs=3))
        ps_tr = ctx.enter_context(tc.tile_pool(name="ps_tr", bufs=2, space="PSUM"))
        ps_mm = ctx.enter_context(tc.tile_pool(name="ps_mm", bufs=2, space="PSUM"))
        ps_out = ctx.enter_context(tc.tile_pool(name="ps_out", bufs=2, space="PSUM"))
        ps_otr = ctx.enter_context(tc.tile_pool(name="ps_otr", bufs=2, space="PSUM"))

        # constants (once per core)
        ident = const_p.tile([128, 128], bf16, tag="ident")
        nc.sync.dma_start(ident[:], id_in[:])
        mask = const_p.tile([128, 896], bf16, tag="mask")
        nc.sync.dma_start(mask[:], mk_in[:])
        w_sb = []
        for k in range(3):
            wt = const_p.tile([128, 3 * H], bf16, tag=f"w{k}")
            nc.sync.dma_start(wt[:], w_in[128 * k:128 * (k + 1), :])
            w_sb.append(wt)

        def emit_body():
          for b in range(BPC):
            # ---- Phase A: load x, cast, transpose to x^T ----
            xt = [xt_p.tile([128, T], bf16, tag="xt", name=f"xt{b}_{k}")
                  for k in range(3)]
            for i in range(NT):
                xn = xn_p.tile([128, E], f32, tag="xn")
                nc.sync.dma_start(xn[:], x_in[b, 128 * i:128 * (i + 1), :])
                xb = xb_p.tile([128, E], bf16, tag="xb")
                nc.vector.tensor_copy(xb[:], xn[:])
                for k in range(3):
                    pst = ps_tr.tile([128, 128], bf16, tag="tr")
                    nc.tensor.transpose(pst[:], xb[:, 128 * k:128 * (k + 1)],
                                        ident[:])
                    nc.vector.tensor_copy(xt[k][:, 128 * i:128 * (i + 1)], pst[:])

            # ---- qkv^T = W^T @ x^T (three M=64 groups, all base partition 0) ----
            qT = qk_p.tile([64, T], bf16, tag="qT")
            kT = qk_p.tile([64, T], bf16, tag="kT")
            vT = vt_p.tile([64, T], bf16, tag="vt")
            dsts = (qT, kT, vT)
            for mt in range(3):
                msl = slice(64 * mt, 64 * (mt + 1))
                for n in range(4):
                    nsl = slice(512 * n, 512 * (n + 1))
                    ps = ps_mm.tile([128, 512], f32, tag="mm")
                    for k in range(3):
                        nc.tensor.matmul(ps[0:64, :], w_sb[k][:, msl],
                                         xt[k][:, nsl],
                                         start=(k == 0), stop=(k == 2))
                    nc.scalar.copy(dsts[mt][:, nsl], ps[0:64, :])

            # ---- v natural [128, 16*65] with ones column per tile ----
            v_nat = vn_p.tile([128, 65 * NT], bf16, tag="vn")
            nc.gpsimd.memset(v_nat[:], 1.0)
            for m in range(NT):
                pst = ps_tr.tile([128, 64], bf16, tag="tr")
                nc.tensor.transpose(pst[:], vT[:, 128 * m:128 * (m + 1)],
                                    ident[0:64, 0:64])
                nc.vector.tensor_copy(v_nat[:, 65 * m:65 * m + 64], pst[:])

            # ---- attention per 512-query chunk ----
            for c in range(NC):
                nm = 4 * c + 4  # key tiles 0..nm-1
                cs = slice(512 * c, 512 * (c + 1))
                ps_o = ps_out.tile([65, 512], f32, tag="out")
                prev_pt = None
                for m in range(nm):
                    ps_s = ps_mm.tile([128, 512], f32, tag="mm")
                    nc.tensor.matmul(ps_s[:, :],
                                     kT[:, 128 * m:128 * (m + 1)],
                                     qT[:, cs], start=True, stop=True)
                    pt = pt_p.tile([128, 512], bf16, tag="pt")
                    nc.scalar.activation(pt[:], ps_s[:, :],
                                         mybir.ActivationFunctionType.Exp,
                                         scale=0.125)
                    d = m - 4 * c
                    if d >= 0:
                        off = 384 - 128 * d
                        nc.vector.tensor_mul(pt[:], pt[:],
                                             mask[:, off:off + 512])
                    # software-pipelined PV (one behind)
                    if prev_pt is not None:
                        pm = m - 1
                        nc.tensor.matmul(ps_o[:, :],
                                         v_nat[:, 65 * pm:65 * pm + 65],
                                         prev_pt[:],
                                         start=(pm == 0), stop=False)
                    prev_pt = pt
                pm = nm - 1
                nc.tensor.matmul(ps_o[:, :], v_nat[:, 65 * pm:65 * pm + 65],
                                 prev_pt[:], start=(pm == 0), stop=True)

                # ---- epilogue: normalize + transpose back + store ----
                otb = ot_p.tile([65, 512], bf16, tag="ot")
                nc.scalar.copy(otb[:], ps_o[:, :])
                for j in range(4):
                    pso = ps_otr.tile([128, 65], bf16, tag="otr")
                    nc.tensor.transpose(pso[:], otb[:, 128 * j:128 * (j + 1)],
                                        ident[0:65, 0:65])
                    rc = rc_p.tile([128, 1], f32, tag="rc")
                    nc.vector.reciprocal(rc[:], pso[:, 64:65])
                    ob = ob_p.tile([128, H], f32, tag="ob")
                    nc.vector.tensor_scalar_mul(ob[:], pso[:, 0:64], rc[:])
                    t0 = 512 * c + 128 * j
                    nc.sync.dma_start(out_t[b, t0:t0 + 128, :], ob[:])

        if loop_n is None:
            emit_body()
        else:
            with tc.For_i(0, loop_n, 1):
                emit_body()

    nc.compile()
    return nc


def _get_nc():
    if "nc" not in _cache:
        _cache["nc"] = _build()
    return _cache["nc"]


def _host_inputs(W_qkv):
    _ensure_path()
    import ml_dtypes

    w_bf = np.ascontiguousarray(W_qkv.astype(ml_dtypes.bfloat16))
    ident = np.eye(128, dtype=ml_dtypes.bfloat16)
    # mask[j, u] = 1 iff u >= j + 384  (sliced per diagonal offset d)
    j = np.arange(128)[:, None]
    u = np.arange(896)[None, :]
    mask = (u >= j + 384).astype(ml_dtypes.bfloat16)
    return w_bf, ident, mask


def kernel(x: np.ndarray, W_qkv: np.ndarray) -> np.ndarray:
    _ensure_path()
    from concourse.bass_utils import run_bass_kernel_spmd

    nc = _get_nc()
    x = np.ascontiguousarray(x, dtype=np.float32)
    w_bf, ident, mask = _host_inputs(np.asarray(W_qkv, dtype=np.float32))
    xs = x.reshape(N_CORES, BPC, T, E)
    in_maps = [
        {"x": xs[c], "w": w_bf, "ident": ident, "mask": mask}
        for c in range(N_CORES)
    ]
    res = run_bass_kernel_spmd(nc, in_maps, list(range(N_CORES)))
    out = np.stack([res.results[c]["out"] for c in range(N_CORES)], axis=0)
    return out.reshape(B, T, H).astype(np.float32)


if __name__ == "__main__":
    rng = np.random.default_rng(0)
    x = rng.standard_normal((B, T, E), dtype=np.float32)
    W = rng.standard_normal((E, 3 * H), dtype=np.float32) * (E ** -0.5)
    out = kernel(x=x, W_qkv=W)
    print("out", out.shape, out.dtype, float(np.abs(out).max()))


# revision 14
# speedup vs baseline: 614.9159x; 1.3492x over previous
"""Single-head causal attention (B=16, T=2048, E=384, H=64) on 8 NeuronCores.

Data-parallel over batch B across the 8 cores (2 batches per core); the tiny
W_qkv is replicated. Hand-written Bass/Tile kernel (bacc.Bacc + TileContext;
nc.compile() runs the wait-splitting legalization that walrus needs).

Per core, per batch:
  1. x tiles [128,384] f32 DMA'd in, cast to bf16 (DVE), PE-transposed to
     x^T bf16 [3x[128,2048]] (PSUM roundtrip).
  2. qkv^T = W^T @ x^T on PE (lhsT = W bf16, three M=64 groups so q^T/k^T/v^T
     all sit at base partition 0), PSUM->SBUF bf16 on DVE.
  3. v^T PE-transposed back to v natural [128, 16*65] with a ones column per
     128-key tile (row-sum trick: PV's 65th output row = softmax denominator).
  4. Per 512-query chunk c, per PAIR of 128-key tiles (one [128,1024] 2-bank
     PSUM span, halving ScalarE instruction count):
       S^T = k_m @ q^T  (keys on partitions, so no P transpose is needed)
       P^T = exp(S^T/8) on ScalarE straight out of PSUM into bf16 SBUF;
       fully-masked left column blocks of diagonal-band tiles are memset 0
       and skipped by exp; triangular band handled by one precomputed
       [128,896] mask multiply (DVE). PV: out^T += v_m^T . P^T accumulated
       in PSUM [65,512], software-pipelined 2 pairs behind exp.
  5. out^T -> bf16, PE-transpose [65,128] blocks -> [128,65], reciprocal of
     col 64 (DVE), tensor_scalar mul, DMA [128,64] f32 tiles to HBM.

Batch 1's phase A is interleaved into batch 0's attention chunks so its
x-loads prefetch during batch 0 compute (and sit ahead of batch 0's output
stores in the DMA queues).

Measured (test.py, marginal per-iteration via on-device For_i loop):
~112 us/core vs ~43 us pure-DMA floor; wall-clock per dispatch through axon
has a fixed ~68 ms floor that dominates any single call.
"""

import sys

import numpy as np

B, T, E, H = 16, 2048, 384, 64
N_CORES = 8
BPC = B // N_CORES  # batches per core
NT = T // 128       # 16 row tiles per batch
NC = T // 512       # 4 query chunks per batch

_cache = {}


def _ensure_path():
    if "/opt/trn_rl_repo" not in sys.path:
        sys.path.insert(0, "/opt/trn_rl_repo")


def _build(loop_n=None, ablate=()):
    """Build + compile the per-core Bass program. Returns nc.

    loop_n: if set, wrap the whole body in an on-device For_i loop that
    repeats the identical computation loop_n times — used by test.py to
    measure marginal per-iteration HW time past the fixed axon dispatch
    floor (~68ms/call).
    ablate: timing-diagnostic stage skips ("phaseA","proj","cast","attn",
    "mask") — results are wrong when used; timing only."""
    _ensure_path()
    from contextlib import ExitStack

    import concourse.bass as bass
    import concourse.tile as tile
    from concourse import bacc, mybir

    f32 = mybir.dt.float32
    bf16 = mybir.dt.bfloat16

    nc = bacc.Bacc("TRN2", target_bir_lowering=False, debug=False,
                   num_devices=N_CORES)

    x_in = nc.declare_dram_parameter("x", [BPC, T, E], f32, isOutput=False)
    w_in = nc.declare_dram_parameter("w", [E, 3 * H], bf16, isOutput=False)
    id_in = nc.declare_dram_parameter("ident", [128, 128], bf16, isOutput=False)
    mk_in = nc.declare_dram_parameter("mask", [128, 896], bf16, isOutput=False)
    out_t = nc.declare_dram_parameter("out", [BPC, T, H], f32, isOutput=True)

    with tile.TileContext(nc) as tc, ExitStack() as ctx:
        const_p = ctx.enter_context(tc.tile_pool(name="const", bufs=1))
        xn_p = ctx.enter_context(tc.tile_pool(name="xn", bufs=18))
        xb_p = ctx.enter_context(tc.tile_pool(name="xb", bufs=6))
        xt_p = ctx.enter_context(tc.tile_pool(name="xt", bufs=6))
        qk_p = ctx.enter_context(tc.tile_pool(name="qk", bufs=2))
        vt_p = ctx.enter_context(tc.tile_pool(name="vt", bufs=2))
        vn_p = ctx.enter_context(tc.tile_pool(name="vn", bufs=2))
        pt_p = ctx.enter_context(tc.tile_pool(name="pt", bufs=8))
        ot_p = ctx.enter_context(tc.tile_pool(name="ot", bufs=2))
        rc_p = ctx.enter_context(tc.tile_pool(name="rc", bufs=3))
        ob_p = ctx.enter_context(tc.tile_pool(name="ob", bufs=3))
        ps_tr = ctx.enter_context(tc.tile_pool(name="ps_tr", bufs=2, space="PSUM"))
        ps_mm = ctx.enter_context(tc.tile_pool(name="ps_mm", bufs=2, space="PSUM"))
        ps_out = ctx.enter_context(tc.tile_pool(name="ps_out", bufs=1, space="PSUM"))
        ps_otr = ctx.enter_context(tc.tile_pool(name="ps_otr", bufs=1, space="PSUM"))

        # constants (once per core)
        ident = const_p.tile([128, 128], bf16, tag="ident")
        nc.sync.dma_start(ident[:], id_in[:])
        mask = const_p.tile([128, 896], bf16, tag="mask")
        nc.sync.dma_start(mask[:], mk_in[:])
        w_sb = []
        for k in range(3):
            wt = const_p.tile([128, 3 * H], bf16, tag=f"w{k}")
            nc.sync.dma_start(wt[:], w_in[128 * k:128 * (k + 1), :])
            w_sb.append(wt)

        def phase_a_tiles(b, xt, i0, i1):
            if "phaseA" in ablate:
                return
            for i in range(i0, i1):
                xn = xn_p.tile([128, E], f32, tag="xn")
                nc.sync.dma_start(xn[:], x_in[b, 128 * i:128 * (i + 1), :])
                if "cast" in ablate:
                    continue
                xb = xb_p.tile([128, E], bf16, tag="xb")
                nc.vector.tensor_copy(xb[:], xn[:])
                if "proj" in ablate:
                    continue
                for k in range(3):
                    pst = ps_tr.tile([128, 128], bf16, tag="tr")
                    nc.tensor.transpose(pst[:], xb[:, 128 * k:128 * (k + 1)],
                                        ident[:])
                    nc.vector.tensor_copy(xt[k][:, 128 * i:128 * (i + 1)],
                                          pst[:])

        def qkv_vnat(b, xt):
            # qkv^T = W^T @ x^T: three M=64 groups, all base partition 0
            do_proj = "phaseA" not in ablate and "proj" not in ablate
            qT = qk_p.tile([64, T], bf16, tag="qT", name=f"qT{b}")
            kT = qk_p.tile([64, T], bf16, tag="kT", name=f"kT{b}")
            vT = vt_p.tile([64, T], bf16, tag="vt", name=f"vT{b}")
            dsts = (qT, kT, vT)
            if not do_proj:
                nc.gpsimd.memset(qT[:], 0.0)
                nc.gpsimd.memset(kT[:], 0.0)
                nc.gpsimd.memset(vT[:], 0.0)
            for mt in range(3 if do_proj else 0):
                msl = slice(64 * mt, 64 * (mt + 1))
                for n in range(4):
                    nsl = slice(512 * n, 512 * (n + 1))
                    ps = ps_mm.tile([128, 512], f32, tag="mm")
                    for k in range(3):
                        nc.tensor.matmul(ps[0:64, :], w_sb[k][:, msl],
                                         xt[k][:, nsl],
                                         start=(k == 0), stop=(k == 2))
                    nc.vector.tensor_copy(dsts[mt][:, nsl], ps[0:64, :])
            # v natural [128, 16*65] with ones column per key tile
            v_nat = vn_p.tile([128, 65 * NT], bf16, tag="vn", name=f"vn{b}")
            nc.gpsimd.memset(v_nat[:], 1.0)
            for m in range(NT if do_proj else 0):
                pst = ps_tr.tile([128, 64], bf16, tag="tr")
                nc.tensor.transpose(pst[:], vT[:, 128 * m:128 * (m + 1)],
                                    ident[0:64, 0:64])
                nc.vector.tensor_copy(v_nat[:, 65 * m:65 * m + 64], pst[:])
            return qT, kT, v_nat

        def attn_chunk(b, c, qT, kT, v_nat):
            nm = 4 * c + 4  # key tiles 0..nm-1
            cs = slice(512 * c, 512 * (c + 1))
            ps_o = ps_out.tile([65, 512], f32, tag="out")
            Exp = mybir.ActivationFunctionType.Exp
            LAGP = 2  # software-pipeline lag, in pairs
            pts = {}
            npair = nm // 2

            def dz(m):
                d = m - 4 * c
                return 0 if d <= 0 else 128 * d

            def pv(pp, stop):
                pt = pts.pop(pp)
                for h in (0, 1):
                    m = 2 * pp + h
                    nc.tensor.matmul(ps_o[:, :],
                                     v_nat[:, 65 * m:65 * m + 65],
                                     pt[:, 512 * h:512 * h + 512],
                                     start=(m == 0), stop=(stop and h == 1))

            for p in range(npair):
                # two S^T key-tiles into one 2-bank PSUM span
                ps_s = ps_mm.tile([128, 1024], f32, tag="mm")
                for h in (0, 1):
                    m = 2 * p + h
                    nc.tensor.matmul(ps_s[:, 512 * h:512 * h + 512],
                                     kT[:, 128 * m:128 * (m + 1)],
                                     qT[:, cs], start=True, stop=True)
                pt = pt_p.tile([128, 1024], bf16, tag="pt")
                if "mask" in ablate:
                    nc.scalar.activation(pt[:], ps_s[:, :], Exp, scale=0.125)
                else:
                    z0, z1 = dz(2 * p), dz(2 * p + 1)
                    # columns < z are fully causally masked: zero them, exp
                    # only the valid span (one exp when the pair is
                    # contiguous, i.e. z1 == 0)
                    if z0 > 0:
                        nc.vector.memset(pt[:, 0:z0], 0.0)
                    if z1 > 0:
                        nc.vector.memset(pt[:, 512:512 + z1], 0.0)
                    if z1 == 0:
                        nc.scalar.activation(pt[:, z0:1024], ps_s[:, z0:1024],
                                             Exp, scale=0.125)
                    else:
                        nc.scalar.activation(pt[:, z0:512], ps_s[:, z0:512],
                                             Exp, scale=0.125)
                        nc.scalar.activation(pt[:, 512 + z1:1024],
                                             ps_s[:, 512 + z1:1024],
                                             Exp, scale=0.125)
                    # triangular band mask on the diagonal part of each half
                    for h, z in ((0, z0), (1, z1)):
                        if 2 * p + h - 4 * c >= 0:
                            lo = 512 * h + z
                            nc.vector.tensor_mul(
                                pt[:, lo:512 * h + 512], pt[:, lo:512 * h + 512],
                                mask[:, 384:384 + 512 - z])
                pts[p] = pt
                if p - LAGP >= 0:
                    pv(p - LAGP, stop=False)
            for pp in range(max(npair - LAGP, 0), npair):
                pv(pp, stop=(pp == npair - 1))
            # epilogue: normalize + transpose back + store
            otb = ot_p.tile([65, 512], bf16, tag="ot")
            nc.vector.tensor_copy(otb[:], ps_o[:, :])
            for j in range(4):
                pso = ps_otr.tile([128, 65], bf16, tag="otr")
                nc.tensor.transpose(pso[:], otb[:, 128 * j:128 * (j + 1)],
                                    ident[0:65, 0:65])
                rc = rc_p.tile([128, 1], f32, tag="rc")
                nc.vector.reciprocal(rc[:], pso[:, 64:65])
                ob = ob_p.tile([128, H], f32, tag="ob")
                nc.vector.tensor_scalar_mul(ob[:], pso[:, 0:64], rc[:])
                t0 = 512 * c + 128 * j
                nc.sync.dma_start(out_t[b, t0:t0 + 128, :], ob[:])

        def emit_body():
            do_attn = "attn" not in ablate
            xt0 = [xt_p.tile([128, T], bf16, tag="xt", name=f"xt0_{k}")
                   for k in range(3)]
            phase_a_tiles(0, xt0, 0, NT)
            st0 = qkv_vnat(0, xt0)
            xt1 = [xt_p.tile([128, T], bf16, tag="xt", name=f"xt1_{k}")
                   for k in range(3)]
            # batch 0 attention interleaved with batch 1 phase A (the b1
            # x-loads land ahead of b0 output stores in the DMA queues, and
            # b1 PE transposes fill exp-wait bubbles)
            for c in range(NC):
                if do_attn:
                    attn_chunk(0, c, *st0)
                phase_a_tiles(1, xt1, 4 * c, 4 * (c + 1))
            st1 = qkv_vnat(1, xt1)
            for c in range(NC if do_attn else 0):
                attn_chunk(1, c, *st1)

        if loop_n is None:
            emit_body()
        else:
            with tc.For_i(0, loop_n, 1):
                emit_body()

    nc.compile()
    return nc


def _get_nc():
    if "nc" not in _cache:
        _cache["nc"] = _build()
    return _cache["nc"]


def _host_inputs(W_qkv):
    _ensure_path()
    import ml_dtypes

    w_bf = np.ascontiguousarray(W_qkv.astype(ml_dtypes.bfloat16))
    ident = np.eye(128, dtype=ml_dtypes.bfloat16)
    # mask[j, u] = 1 iff u >= j + 384  (sliced per diagonal offset d)
    j = np.arange(128)[:, None]
    u = np.arange(896)[None, :]
    mask = (u >= j + 384).astype(ml_dtypes.bfloat16)
    return w_bf, ident, mask


def kernel(x: np.ndarray, W_qkv: np.ndarray) -> np.ndarray:
    _ensure_path()
    from concourse.bass_utils import run_bass_kernel_spmd

    nc = _get_nc()
    x = np.ascontiguousarray(x, dtype=np.float32)
    w_bf, ident, mask = _host_inputs(np.asarray(W_qkv, dtype=np.float32))
    xs = x.reshape(N_CORES, BPC, T, E)
    in_maps = [
        {"x": xs[c], "w": w_bf, "ident": ident, "mask": mask}
        for c in range(N_CORES)
    ]
    res = run_bass_kernel_spmd(nc, in_maps, list(range(N_CORES)))
    out = np.stack([res.results[c]["out"] for c in range(N_CORES)], axis=0)
    return out.reshape(B, T, H).astype(np.float32)


if __name__ == "__main__":
    rng = np.random.default_rng(0)
    x = rng.standard_normal((B, T, E), dtype=np.float32)
    W = rng.standard_normal((E, 3 * H), dtype=np.float32) * (E ** -0.5)
    out = kernel(x=x, W_qkv=W)
    print("out", out.shape, out.dtype, float(np.abs(out).max()))
